# revision 4
# baseline (speedup 1.0000x reference)
"""Trainium2 Bass kernel for single-head attention (N=16384, F=512, M=128),
sequence-parallel over 8 NeuronCores.

Strategy (hardcoded, self-contained):
- Each core owns 2048 query rows. K/V are computed redundantly on every core
  (cheap projections vs. the O(N^2) attention math) -> no collectives.
- Host passes x^T (bf16) per core, rotated so the core's own query columns
  are always columns 0:2048 -> identical SPMD graph on all cores. Softmax
  sums over keys are permutation-invariant, so rotated K/V order is harmless.
- All matmuls run in bf16 (PE full rate), accumulation in fp32 PSUM.
- Scores are computed transposed (S^T = K @ Q^T, layout [j, q]) so the exp
  output E^T feeds the E^T.T @ V matmul directly with no transposes.
- Softmax denominators via ones-vector matmul accumulated in PSUM; the
  final 1/sum scaling is folded past the (linear) output projection and
  applied per-partition on the [q, f] output tiles.
"""

import math
import sys

import numpy as np

for _p in ("/opt/trn_rl_repo", "/opt/pypackages"):
    if _p not in sys.path:
        sys.path.append(_p)

import ml_dtypes

N = 16384
F = 512
MD = 128
P = 128
NCORES = 8
NQ = N // NCORES      # 2048 query rows per core
QB = 512              # q-block (one PSUM bank of fp32)
NQB = NQ // QB        # 4
JT = 128              # j (key) tile
NJT = N // JT         # 128
FK = F // P           # 4 contraction tiles over features
CH = 512              # xt streaming chunk (j columns)
NCH = N // CH         # 32
GK = 16               # j-tiles per SBUF super-group
NG = NJT // GK        # 8
SCALE = 1.0 / math.sqrt(MD)

_BF16 = ml_dtypes.bfloat16


def _build():
    import concourse.bass as bass  # noqa: F401
    import concourse.tile as tile
    from concourse import bacc, mybir

    f32 = mybir.dt.float32
    bf16 = mybir.dt.bfloat16
    AF = mybir.ActivationFunctionType
    ALU = mybir.AluOpType

    nc = bacc.Bacc("TRN2", target_bir_lowering=False, debug=False,
                   num_devices=NCORES)

    xt = nc.declare_dram_parameter("xt", [F, N], bf16, isOutput=False)
    wq = nc.declare_dram_parameter("wq", [F, MD], bf16, isOutput=False)
    wk = nc.declare_dram_parameter("wk", [F, MD], bf16, isOutput=False)
    wv = nc.declare_dram_parameter("wv", [F, F], bf16, isOutput=False)
    wo = nc.declare_dram_parameter("wo", [F, F], bf16, isOutput=False)
    bq = nc.declare_dram_parameter("bq", [MD, 1], f32, isOutput=False)
    bk = nc.declare_dram_parameter("bk", [MD, 1], f32, isOutput=False)
    bv = nc.declare_dram_parameter("bv", [1, F], f32, isOutput=False)
    bo = nc.declare_dram_parameter("bo", [1, F], f32, isOutput=False)
    out = nc.declare_dram_parameter("out", [NQ, F], f32, isOutput=True)

    with tile.TileContext(nc) as tc:
        with (
            tc.tile_pool(name="persist", bufs=1) as pp,
            tc.tile_pool(name="stream", bufs=2) as sp,
            tc.tile_pool(name="work", bufs=3) as wkp,
            tc.tile_pool(name="pssc", bufs=2, space="PSUM") as ps_sc,
            tc.tile_pool(name="pso", bufs=4, space="PSUM") as ps_o,
            tc.tile_pool(name="pssf", bufs=2, space="PSUM") as ps_sf,
        ):
            # ---- persistent constants -------------------------------------
            wq_t = [pp.tile([P, MD], bf16, tag=f"wq{k}", name=f"wq{k}") for k in range(FK)]
            wk_t = [pp.tile([P, MD], bf16, tag=f"wk{k}", name=f"wk{k}") for k in range(FK)]
            wv_t = [pp.tile([P, F], bf16, tag=f"wv{k}", name=f"wv{k}") for k in range(FK)]
            wo_t = [pp.tile([P, F], bf16, tag=f"wo{k}", name=f"wo{k}") for k in range(FK)]
            for k in range(FK):
                nc.sync.dma_start(out=wq_t[k][:], in_=wq[k * P:(k + 1) * P, :])
                nc.sync.dma_start(out=wk_t[k][:], in_=wk[k * P:(k + 1) * P, :])
                nc.sync.dma_start(out=wv_t[k][:], in_=wv[k * P:(k + 1) * P, :])
                nc.sync.dma_start(out=wo_t[k][:], in_=wo[k * P:(k + 1) * P, :])
            bq_t = pp.tile([MD, 1], f32, tag="bq")
            bk_t = pp.tile([MD, 1], f32, tag="bk")
            nc.sync.dma_start(out=bq_t[:], in_=bq[:])
            nc.sync.dma_start(out=bk_t[:], in_=bk[:])
            bv_r = pp.tile([P, F], f32, tag="bvr")
            bo_r = pp.tile([P, F], f32, tag="bor")
            nc.sync.dma_start(out=bv_r[:], in_=bv[:].to_broadcast((P, F)))
            nc.sync.dma_start(out=bo_r[:], in_=bo[:].to_broadcast((P, F)))
            ones_b = pp.tile([P, 1], bf16, tag="ones")
            nc.vector.memset(ones_b[:], 1.0)
            one_f = pp.tile([1, 1], f32, tag="onef")
            nc.vector.memset(one_f[:], 1.0)

            # ---- persistent activations -----------------------------------
            # K^T grouped [md, j]: NG groups of [P, GK*JT]
            ktg = [pp.tile([P, GK * JT], bf16, tag=f"ktg{g}", name=f"ktg{g}") for g in range(NG)]
            # V grouped [j, f]: per group GK j-tiles each [P, F]
            vg = [pp.tile([P, GK * F], bf16, tag=f"vg{g}", name=f"vg{g}") for g in range(NG)]
            # Q^T [md, q] for this core's 2048 queries
            qt = pp.tile([P, NQ], bf16, tag="qt")

            # ---- prologue: project Q^T, K^T, V from streamed x^T ----------
            for ch in range(NCH):
                xt_t = [sp.tile([P, CH], bf16, tag=f"xt{k}", name=f"xt{k}") for k in range(FK)]
                for k in range(FK):
                    nc.sync.dma_start(
                        out=xt_t[k][:],
                        in_=xt[k * P:(k + 1) * P, ch * CH:(ch + 1) * CH])
                g, off = ch // 4, (ch % 4) * CH
                # K^T chunk [md, CH]
                pk = ps_sc.tile([P, CH], f32, tag="sc", name="pk")
                for k in range(FK):
                    nc.tensor.matmul(pk[:], wk_t[k][:], xt_t[k][:],
                                     start=(k == 0), stop=(k == FK - 1))
                nc.scalar.activation(ktg[g][:, off:off + CH], pk[:],
                                     AF.Identity, bias=bk_t[:], scale=1.0)
                # Q^T chunk (the first NQ columns are this core's queries)
                if ch < NQ // CH:
                    pq = ps_sc.tile([P, CH], f32, tag="sc", name="pq")
                    for k in range(FK):
                        nc.tensor.matmul(pq[:], wq_t[k][:], xt_t[k][:],
                                         start=(k == 0), stop=(k == FK - 1))
                    nc.scalar.activation(qt[:, ch * CH:(ch + 1) * CH], pq[:],
                                         AF.Identity, bias=bq_t[:], scale=1.0)
                # V tiles [j, f]
                for js in range(CH // JT):
                    jt_g = ch * (CH // JT) + js
                    voff = (jt_g % GK) * F
                    pv = ps_sc.tile([P, F], f32, tag="sc", name="pv")
                    for k in range(FK):
                        nc.tensor.matmul(
                            pv[:], xt_t[k][:, js * JT:(js + 1) * JT], wv_t[k][:],
                            start=(k == 0), stop=(k == FK - 1))
                    nc.vector.tensor_tensor(
                        vg[jt_g // GK][:, voff:voff + F], pv[:], bv_r[:], ALU.add)

            # ---- attention: per q-block sweep over all keys ---------------
            for qb in range(NQB):
                q_sl = qt[:, qb * QB:(qb + 1) * QB]
                ps_sums = ps_sf.tile([1, QB], f32, tag="sf", name="ps_sums")
                po = [ps_o.tile([P, QB], f32, tag="oacc", name="oacc") for _ in range(FK)]
                for jt_i in range(NJT):
                    g, r = jt_i // GK, jt_i % GK
                    psc = ps_sc.tile([P, QB], f32, tag="sc")
                    nc.tensor.matmul(psc[:], ktg[g][:, r * JT:(r + 1) * JT],
                                     q_sl, start=True, stop=True)
                    et = wkp.tile([P, QB], bf16, tag="et", bufs=3)
                    nc.scalar.activation(et[:], psc[:], AF.Exp, scale=SCALE)
                    nc.tensor.matmul(ps_sums[:], ones_b[:], et[:],
                                     start=(jt_i == 0), stop=(jt_i == NJT - 1))
                    for ft in range(FK):
                        nc.tensor.matmul(
                            po[ft][:],
                            vg[g][:, r * F + ft * P:r * F + (ft + 1) * P],
                            et[:], start=(jt_i == 0), stop=(jt_i == NJT - 1))
                # epilogue: 1/sums, transpose to per-partition, project, store
                recip_s = wkp.tile([1, QB], f32, tag="recip", bufs=2)
                nc.vector.reciprocal(recip_s[:], ps_sums[:])
                ot = wkp.tile([P, FK * QB], bf16, tag="ot", bufs=1)
                for ft in range(FK):
                    nc.vector.tensor_copy(ot[:, ft * QB:(ft + 1) * QB], po[ft][:])
                recip_p = wkp.tile([P, QB // P], f32, tag="recipp", bufs=2)
                for qs in range(QB // P):
                    pt = ps_sf.tile([P, 1], f32, tag="sf", name="pt")
                    nc.tensor.matmul(pt[:], recip_s[:, qs * P:(qs + 1) * P],
                                     one_f[:], start=True, stop=True)
                    nc.scalar.copy(recip_p[:, qs:qs + 1], pt[:])
                    pf = ps_sf.tile([P, F], f32, tag="sf", name="pf")
                    for ft in range(FK):
                        nc.tensor.matmul(
                            pf[:], ot[:, ft * QB + qs * P:ft * QB + (qs + 1) * P],
                            wo_t[ft][:], start=(ft == 0), stop=(ft == FK - 1))
                    out_t = wkp.tile([P, F], f32, tag="outt", bufs=2)
                    nc.vector.scalar_tensor_tensor(
                        out_t[:], pf[:], recip_p[:, qs:qs + 1], bo_r[:],
                        ALU.mult, ALU.add)
                    row0 = qb * QB + qs * P
                    nc.sync.dma_start(out=out[row0:row0 + P, :], in_=out_t[:])

    nc.compile()
    return nc


_CACHED = {}


def _get_nc():
    if "nc" not in _CACHED:
        _CACHED["nc"] = _build()
    return _CACHED["nc"]


def kernel(x, Wq, bq, Wk, bk, Wv, bv, Wo, bo):
    from concourse.bass_utils import run_bass_kernel_spmd

    x = np.asarray(x, dtype=np.float32)
    xt_full = np.ascontiguousarray(x.T)                      # [F, N] f32
    wq_b = np.asarray(Wq, dtype=np.float32).astype(_BF16)
    wk_b = np.asarray(Wk, dtype=np.float32).astype(_BF16)
    wv_b = np.asarray(Wv, dtype=np.float32).astype(_BF16)
    wo_b = np.asarray(Wo, dtype=np.float32).astype(_BF16)
    bq_h = np.asarray(bq, dtype=np.float32).reshape(MD, 1)
    bk_h = np.asarray(bk, dtype=np.float32).reshape(MD, 1)
    bv_h = np.asarray(bv, dtype=np.float32).reshape(1, F)
    bo_h = np.asarray(bo, dtype=np.float32).reshape(1, F)

    in_maps = []
    for c in range(NCORES):
        s = c * NQ
        xt_rot = np.concatenate([xt_full[:, s:], xt_full[:, :s]], axis=1)
        in_maps.append({
            "xt": np.ascontiguousarray(xt_rot).astype(_BF16),
            "wq": wq_b, "wk": wk_b, "wv": wv_b, "wo": wo_b,
            "bq": bq_h, "bk": bk_h, "bv": bv_h, "bo": bo_h,
        })

    nc = _get_nc()
    res = run_bass_kernel_spmd(nc, in_maps, core_ids=list(range(NCORES)))
    return np.concatenate(
        [res.results[c]["out"] for c in range(NCORES)], axis=0)


def run_traced(x, Wq, bq, Wk, bk, Wv, bv, Wo, bo):
    """Like kernel() but with NTFF tracing; returns (output, exec_time_ns)."""
    from concourse.bass_utils import run_bass_kernel_spmd

    try:
        import ntff_shim
        ntff_shim.install()
    except ImportError:
        pass
    x = np.asarray(x, dtype=np.float32)
    xt_full = np.ascontiguousarray(x.T)
    in_maps = []
    for c in range(NCORES):
        s = c * NQ
        xt_rot = np.concatenate([xt_full[:, s:], xt_full[:, :s]], axis=1)
        in_maps.append({
            "xt": np.ascontiguousarray(xt_rot).astype(_BF16),
            "wq": np.asarray(Wq, np.float32).astype(_BF16),
            "wk": np.asarray(Wk, np.float32).astype(_BF16),
            "wv": np.asarray(Wv, np.float32).astype(_BF16),
            "wo": np.asarray(Wo, np.float32).astype(_BF16),
            "bq": np.asarray(bq, np.float32).reshape(MD, 1),
            "bk": np.asarray(bk, np.float32).reshape(MD, 1),
            "bv": np.asarray(bv, np.float32).reshape(1, F),
            "bo": np.asarray(bo, np.float32).reshape(1, F),
        })
    nc = _get_nc()
    res = run_bass_kernel_spmd(nc, in_maps, core_ids=list(range(NCORES)),
                               trace=True)
    out = np.concatenate([res.results[c]["out"] for c in range(NCORES)], axis=0)
    return out, res.exec_time_ns


# revision 5
# speedup vs baseline: 1.5044x; 1.5044x over previous
"""Trainium2 Bass kernel for single-head attention (N=16384, F=512, M=128),
sequence-parallel over 8 NeuronCores.

Strategy (hardcoded, self-contained):
- Each core owns 2048 query rows. K/V are computed redundantly on every core
  (cheap projections vs. the O(N^2) attention math) -> no collectives.
- Host passes x^T (bf16) per core, rotated so the core's own query columns
  are always columns 0:2048 -> identical SPMD graph on all cores. Softmax
  sums over keys are permutation-invariant, so rotated K/V order is harmless.
- All matmuls run in bf16 (PE full rate), accumulation in fp32 PSUM.
- Scores are computed transposed (S^T = K @ Q^T, layout [j, q]) so the exp
  output E^T feeds the E^T.T @ V matmul directly with no transposes.
- Softmax denominators via ones-vector matmul accumulated in PSUM; the
  final 1/sum scaling is folded past the (linear) output projection and
  applied per-partition on the [q, f] output tiles.
"""

import math
import sys

import numpy as np

for _p in ("/opt/trn_rl_repo", "/opt/pypackages"):
    if _p not in sys.path:
        sys.path.append(_p)

import ml_dtypes

N = 16384
F = 512
MD = 128
P = 128
NCORES = 8
NQ = N // NCORES      # 2048 query rows per core
QB = 512              # q-block (one PSUM bank of fp32)
NQB = NQ // QB        # 4
JT = 128              # j (key) tile
NJT = N // JT         # 128
FK = F // P           # 4 contraction tiles over features
CH = 512              # xt streaming chunk (j columns)
NCH = N // CH         # 32
GK = 16               # j-tiles per SBUF super-group
NG = NJT // GK        # 8
SCALE = 1.0 / math.sqrt(MD)

_BF16 = ml_dtypes.bfloat16


def _build():
    import concourse.bass as bass  # noqa: F401
    import concourse.tile as tile
    from concourse import bacc, mybir

    f32 = mybir.dt.float32
    bf16 = mybir.dt.bfloat16
    AF = mybir.ActivationFunctionType
    ALU = mybir.AluOpType

    nc = bacc.Bacc("TRN2", target_bir_lowering=False, debug=False,
                   num_devices=NCORES)

    xt = nc.declare_dram_parameter("xt", [F, N], bf16, isOutput=False)
    wq = nc.declare_dram_parameter("wq", [F, MD], bf16, isOutput=False)
    wk = nc.declare_dram_parameter("wk", [F, MD], bf16, isOutput=False)
    wv = nc.declare_dram_parameter("wv", [F, F], bf16, isOutput=False)
    wo = nc.declare_dram_parameter("wo", [F, F], bf16, isOutput=False)
    bq = nc.declare_dram_parameter("bq", [MD, 1], f32, isOutput=False)
    bk = nc.declare_dram_parameter("bk", [MD, 1], f32, isOutput=False)
    bv = nc.declare_dram_parameter("bv", [1, F], f32, isOutput=False)
    bo = nc.declare_dram_parameter("bo", [1, F], f32, isOutput=False)
    out = nc.declare_dram_parameter("out", [NQ, F], f32, isOutput=True)

    with tile.TileContext(nc) as tc:
        with (
            tc.tile_pool(name="persist", bufs=1) as pp,
            tc.tile_pool(name="stream", bufs=2) as sp,
            tc.tile_pool(name="work", bufs=3) as wkp,
            tc.tile_pool(name="pssc", bufs=3, space="PSUM") as ps_sc,
            tc.tile_pool(name="pso", bufs=4, space="PSUM") as ps_o,
            tc.tile_pool(name="pssf", bufs=1, space="PSUM") as ps_sf,
        ):
            # ---- persistent constants -------------------------------------
            wq_t = [pp.tile([P, MD], bf16, tag=f"wq{k}", name=f"wq{k}") for k in range(FK)]
            wk_t = [pp.tile([P, MD], bf16, tag=f"wk{k}", name=f"wk{k}") for k in range(FK)]
            wv_t = [pp.tile([P, F], bf16, tag=f"wv{k}", name=f"wv{k}") for k in range(FK)]
            wo_t = [pp.tile([P, F], bf16, tag=f"wo{k}", name=f"wo{k}") for k in range(FK)]
            for k in range(FK):
                nc.sync.dma_start(out=wq_t[k][:], in_=wq[k * P:(k + 1) * P, :])
                nc.sync.dma_start(out=wk_t[k][:], in_=wk[k * P:(k + 1) * P, :])
                nc.sync.dma_start(out=wv_t[k][:], in_=wv[k * P:(k + 1) * P, :])
                nc.sync.dma_start(out=wo_t[k][:], in_=wo[k * P:(k + 1) * P, :])
            bq_t = pp.tile([MD, 1], f32, tag="bq")
            bk_t = pp.tile([MD, 1], f32, tag="bk")
            nc.sync.dma_start(out=bq_t[:], in_=bq[:])
            nc.sync.dma_start(out=bk_t[:], in_=bk[:])
            bv_r = pp.tile([P, F], f32, tag="bvr")
            bo_r = pp.tile([P, F], f32, tag="bor")
            nc.sync.dma_start(out=bv_r[:], in_=bv[:].to_broadcast((P, F)))
            nc.sync.dma_start(out=bo_r[:], in_=bo[:].to_broadcast((P, F)))
            ones_f = pp.tile([P, 1], f32, tag="ones")
            nc.vector.memset(ones_f[:], 1.0)

            # ---- persistent activations -----------------------------------
            # K^T grouped [md, j]: NG groups of [P, GK*JT]
            ktg = [pp.tile([P, GK * JT], bf16, tag=f"ktg{g}", name=f"ktg{g}") for g in range(NG)]
            # V grouped [j, f]: per group GK j-tiles each [P, F]
            vg = [pp.tile([P, GK * F], bf16, tag=f"vg{g}", name=f"vg{g}") for g in range(NG)]
            # Q^T [md, q] for this core's 2048 queries
            qt = pp.tile([P, NQ], bf16, tag="qt")

            # ---- prologue: project Q^T, K^T, V from streamed x^T ----------
            for ch in range(NCH):
                xt_t = [sp.tile([P, CH], bf16, tag=f"xt{k}", name=f"xt{k}") for k in range(FK)]
                for k in range(FK):
                    nc.gpsimd.dma_start(
                        out=xt_t[k][:],
                        in_=xt[k * P:(k + 1) * P, ch * CH:(ch + 1) * CH])
                g, off = ch // 4, (ch % 4) * CH
                # K^T chunk [md, CH]
                pk = ps_sc.tile([P, CH], f32, tag="sc", name="pk")
                for k in range(FK):
                    nc.tensor.matmul(pk[:], wk_t[k][:], xt_t[k][:],
                                     start=(k == 0), stop=(k == FK - 1))
                nc.scalar.activation(ktg[g][:, off:off + CH], pk[:],
                                     AF.Identity, bias=bk_t[:], scale=1.0)
                # Q^T chunk (the first NQ columns are this core's queries)
                if ch < NQ // CH:
                    pq = ps_sc.tile([P, CH], f32, tag="sc", name="pq")
                    for k in range(FK):
                        nc.tensor.matmul(pq[:], wq_t[k][:], xt_t[k][:],
                                         start=(k == 0), stop=(k == FK - 1))
                    nc.scalar.activation(qt[:, ch * CH:(ch + 1) * CH], pq[:],
                                         AF.Identity, bias=bq_t[:], scale=1.0)
                # V tiles [j, f]
                for js in range(CH // JT):
                    jt_g = ch * (CH // JT) + js
                    voff = (jt_g % GK) * F
                    pv = ps_sc.tile([P, F], f32, tag="sc", name="pv")
                    for k in range(FK):
                        nc.tensor.matmul(
                            pv[:], xt_t[k][:, js * JT:(js + 1) * JT], wv_t[k][:],
                            start=(k == 0), stop=(k == FK - 1))
                    nc.vector.tensor_tensor(
                        vg[jt_g // GK][:, voff:voff + F], pv[:], bv_r[:], ALU.add)

            # ---- attention: per q-block sweep over all keys ---------------
            for qb in range(NQB):
                q_sl = qt[:, qb * QB:(qb + 1) * QB]
                po = [ps_o.tile([P, QB], f32, tag="oacc", name="oacc") for _ in range(FK)]
                acc = [wkp.tile([P, QB], f32, tag="acc", bufs=4, name=f"acc{a}")
                       for a in range(2)]

                def scores(jt_i):
                    g, r = jt_i // GK, jt_i % GK
                    psc = ps_sc.tile([P, QB], f32, tag="sc", name="psc")
                    nc.tensor.matmul(psc[:], ktg[g][:, r * JT:(r + 1) * JT],
                                     q_sl, start=True, stop=True)
                    return psc

                pending = {0: scores(0), 1: scores(1)}
                for jt_i in range(NJT):
                    g, r = jt_i // GK, jt_i % GK
                    psc = pending.pop(jt_i)
                    et = wkp.tile([P, QB], bf16, tag="et", bufs=4)
                    nc.scalar.activation(et[:], psc[:], AF.Exp, scale=SCALE)
                    if jt_i + 2 < NJT:
                        pending[jt_i + 2] = scores(jt_i + 2)
                    # row-sum accumulation on DVE (two chains to halve latency)
                    a = jt_i % 2
                    if jt_i < 2:
                        nc.vector.tensor_copy(acc[a][:], et[:])
                    else:
                        nc.vector.tensor_tensor(acc[a][:], acc[a][:], et[:],
                                                ALU.add)
                    for ft in range(FK):
                        nc.tensor.matmul(
                            po[ft][:],
                            vg[g][:, r * F + ft * P:r * F + (ft + 1) * P],
                            et[:], start=(jt_i == 0), stop=(jt_i == NJT - 1))
                # epilogue: per-partition 1/sums, project, scale+bias, store
                ot = wkp.tile([P, FK * QB], bf16, tag="ot", bufs=1)
                for ft in range(FK):
                    nc.vector.tensor_copy(ot[:, ft * QB:(ft + 1) * QB], po[ft][:])
                recip_p = wkp.tile([P, QB // P], f32, tag="recipp", bufs=2)
                for qs in range(QB // P):
                    pt = ps_sf.tile([P, 1], f32, tag="sf", name="pt")
                    nc.tensor.matmul(pt[:], acc[0][:, qs * P:(qs + 1) * P],
                                     ones_f[:], start=True, stop=False)
                    nc.tensor.matmul(pt[:], acc[1][:, qs * P:(qs + 1) * P],
                                     ones_f[:], start=False, stop=True)
                    nc.vector.reciprocal(recip_p[:, qs:qs + 1], pt[:])
                    pf = ps_sf.tile([P, F], f32, tag="sf", name="pf")
                    for ft in range(FK):
                        nc.tensor.matmul(
                            pf[:], ot[:, ft * QB + qs * P:ft * QB + (qs + 1) * P],
                            wo_t[ft][:], start=(ft == 0), stop=(ft == FK - 1))
                    out_t = wkp.tile([P, F], f32, tag="outt", bufs=2)
                    nc.vector.scalar_tensor_tensor(
                        out_t[:], pf[:], recip_p[:, qs:qs + 1], bo_r[:],
                        ALU.mult, ALU.add)
                    row0 = qb * QB + qs * P
                    nc.sync.dma_start(out=out[row0:row0 + P, :], in_=out_t[:])

    nc.compile()
    return nc


_CACHED = {}


def _get_nc():
    if "nc" not in _CACHED:
        _CACHED["nc"] = _build()
    return _CACHED["nc"]


def kernel(x, Wq, bq, Wk, bk, Wv, bv, Wo, bo):
    from concourse.bass_utils import run_bass_kernel_spmd

    x = np.asarray(x, dtype=np.float32)
    xt_full = np.ascontiguousarray(x.T)                      # [F, N] f32
    wq_b = np.asarray(Wq, dtype=np.float32).astype(_BF16)
    wk_b = np.asarray(Wk, dtype=np.float32).astype(_BF16)
    wv_b = np.asarray(Wv, dtype=np.float32).astype(_BF16)
    wo_b = np.asarray(Wo, dtype=np.float32).astype(_BF16)
    bq_h = np.asarray(bq, dtype=np.float32).reshape(MD, 1)
    bk_h = np.asarray(bk, dtype=np.float32).reshape(MD, 1)
    bv_h = np.asarray(bv, dtype=np.float32).reshape(1, F)
    bo_h = np.asarray(bo, dtype=np.float32).reshape(1, F)

    in_maps = []
    for c in range(NCORES):
        s = c * NQ
        xt_rot = np.concatenate([xt_full[:, s:], xt_full[:, :s]], axis=1)
        in_maps.append({
            "xt": np.ascontiguousarray(xt_rot).astype(_BF16),
            "wq": wq_b, "wk": wk_b, "wv": wv_b, "wo": wo_b,
            "bq": bq_h, "bk": bk_h, "bv": bv_h, "bo": bo_h,
        })

    nc = _get_nc()
    res = run_bass_kernel_spmd(nc, in_maps, core_ids=list(range(NCORES)))
    return np.concatenate(
        [res.results[c]["out"] for c in range(NCORES)], axis=0)


def run_traced(x, Wq, bq, Wk, bk, Wv, bv, Wo, bo):
    """Like kernel() but with NTFF tracing; returns (output, exec_time_ns)."""
    from concourse.bass_utils import run_bass_kernel_spmd

    try:
        import ntff_shim
        ntff_shim.install()
    except ImportError:
        pass
    x = np.asarray(x, dtype=np.float32)
    xt_full = np.ascontiguousarray(x.T)
    in_maps = []
    for c in range(NCORES):
        s = c * NQ
        xt_rot = np.concatenate([xt_full[:, s:], xt_full[:, :s]], axis=1)
        in_maps.append({
            "xt": np.ascontiguousarray(xt_rot).astype(_BF16),
            "wq": np.asarray(Wq, np.float32).astype(_BF16),
            "wk": np.asarray(Wk, np.float32).astype(_BF16),
            "wv": np.asarray(Wv, np.float32).astype(_BF16),
            "wo": np.asarray(Wo, np.float32).astype(_BF16),
            "bq": np.asarray(bq, np.float32).reshape(MD, 1),
            "bk": np.asarray(bk, np.float32).reshape(MD, 1),
            "bv": np.asarray(bv, np.float32).reshape(1, F),
            "bo": np.asarray(bo, np.float32).reshape(1, F),
        })
    nc = _get_nc()
    res = run_bass_kernel_spmd(nc, in_maps, core_ids=list(range(NCORES)),
                               trace=True)
    out = np.concatenate([res.results[c]["out"] for c in range(NCORES)], axis=0)
    return out, res.exec_time_ns


# revision 6
# speedup vs baseline: 2.0886x; 1.3883x over previous
"""Trainium2 Bass kernel for single-head attention (N=16384, F=512, M=128),
sequence-parallel over 8 NeuronCores.

Strategy (hardcoded, self-contained):
- Each core owns 2048 query rows. K/V are computed redundantly on every core
  (cheap projections vs. the O(N^2) attention math) -> no collectives.
- Host passes x^T (bf16) per core, rotated so the core's own query columns
  are always columns 0:2048 -> identical SPMD graph on all cores. Softmax
  sums over keys are permutation-invariant, so rotated K/V order is harmless.
- All matmuls run in bf16 (PE full rate), accumulation in fp32 PSUM.
- Scores are computed transposed (S^T = K @ Q^T, layout [j, q]) so the exp
  output E^T feeds the E^T.T @ V matmul directly with no transposes.
- Softmax denominators via ones-vector matmul accumulated in PSUM; the
  final 1/sum scaling is folded past the (linear) output projection and
  applied per-partition on the [q, f] output tiles.
"""

import math
import sys

import numpy as np

for _p in ("/opt/trn_rl_repo", "/opt/pypackages"):
    if _p not in sys.path:
        sys.path.append(_p)

import ml_dtypes

N = 16384
F = 512
MD = 128
P = 128
NCORES = 8
NQ = N // NCORES      # 2048 query rows per core
QB = 512              # q-block (one PSUM bank of fp32)
NQB = NQ // QB        # 4
JT = 128              # j (key) tile
NJT = N // JT         # 128
FK = F // P           # 4 contraction tiles over features
CH = 512              # xt streaming chunk (j columns)
NCH = N // CH         # 32
GK = 16               # j-tiles per SBUF super-group
NG = NJT // GK        # 8
SCALE = 1.0 / math.sqrt(MD)

_BF16 = ml_dtypes.bfloat16


def _build():
    import concourse.bass as bass  # noqa: F401
    import concourse.tile as tile
    from concourse import bacc, mybir

    f32 = mybir.dt.float32
    bf16 = mybir.dt.bfloat16
    fp8 = mybir.dt.float8e4
    DR = mybir.MatmulPerfMode.DoubleRow
    AF = mybir.ActivationFunctionType
    ALU = mybir.AluOpType

    nc = bacc.Bacc("TRN2", target_bir_lowering=False, debug=False,
                   num_devices=NCORES)

    xt = nc.declare_dram_parameter("xt", [F, N], bf16, isOutput=False)
    wq = nc.declare_dram_parameter("wq", [F, MD], bf16, isOutput=False)
    wk = nc.declare_dram_parameter("wk", [F, MD], bf16, isOutput=False)
    wv = nc.declare_dram_parameter("wv", [F, F], bf16, isOutput=False)
    wo = nc.declare_dram_parameter("wo", [F, F], bf16, isOutput=False)
    bq = nc.declare_dram_parameter("bq", [MD, 1], f32, isOutput=False)
    bk = nc.declare_dram_parameter("bk", [MD, 1], f32, isOutput=False)
    bv = nc.declare_dram_parameter("bv", [1, F], f32, isOutput=False)
    bo = nc.declare_dram_parameter("bo", [1, F], f32, isOutput=False)
    out = nc.declare_dram_parameter("out", [NQ, F], f32, isOutput=True)

    with tile.TileContext(nc) as tc:
        with (
            tc.tile_pool(name="persist", bufs=1) as pp,
            tc.tile_pool(name="stream", bufs=2) as sp,
            tc.tile_pool(name="work", bufs=3) as wkp,
            tc.tile_pool(name="pssc", bufs=3, space="PSUM") as ps_sc,
            tc.tile_pool(name="pso", bufs=4, space="PSUM") as ps_o,
            tc.tile_pool(name="pssf", bufs=1, space="PSUM") as ps_sf,
        ):
            # ---- persistent constants -------------------------------------
            wq_t = [pp.tile([P, MD], bf16, tag=f"wq{k}", name=f"wq{k}") for k in range(FK)]
            wk_t = [pp.tile([P, MD], bf16, tag=f"wk{k}", name=f"wk{k}") for k in range(FK)]
            wv_t = [pp.tile([P, F], bf16, tag=f"wv{k}", name=f"wv{k}") for k in range(FK)]
            wo_t = [pp.tile([P, F], bf16, tag=f"wo{k}", name=f"wo{k}") for k in range(FK)]
            for k in range(FK):
                nc.sync.dma_start(out=wq_t[k][:], in_=wq[k * P:(k + 1) * P, :])
                nc.sync.dma_start(out=wk_t[k][:], in_=wk[k * P:(k + 1) * P, :])
                nc.sync.dma_start(out=wv_t[k][:], in_=wv[k * P:(k + 1) * P, :])
                nc.sync.dma_start(out=wo_t[k][:], in_=wo[k * P:(k + 1) * P, :])
            bq_t = pp.tile([MD, 1], f32, tag="bq")
            bk_t = pp.tile([MD, 1], f32, tag="bk")
            nc.sync.dma_start(out=bq_t[:], in_=bq[:])
            nc.sync.dma_start(out=bk_t[:], in_=bk[:])
            bv_r = pp.tile([P, F], f32, tag="bvr")
            bo_r = pp.tile([P, F], f32, tag="bor")
            nc.sync.dma_start(out=bv_r[:], in_=bv[:].to_broadcast((P, F)))
            nc.sync.dma_start(out=bo_r[:], in_=bo[:].to_broadcast((P, F)))
            ones_f = pp.tile([P, 1], f32, tag="ones")
            nc.vector.memset(ones_f[:], 1.0)

            # ---- persistent activations -----------------------------------
            # K^T grouped [md, j]: NG groups of [P, GK*JT]
            ktg = [pp.tile([P, GK * JT], bf16, tag=f"ktg{g}", name=f"ktg{g}") for g in range(NG)]
            # V grouped [j, f]: per group GK j-tiles each [P, F]
            vg = [pp.tile([P, GK * F], fp8, tag=f"vg{g}", name=f"vg{g}") for g in range(NG)]
            # Q^T [md, q] for this core's 2048 queries
            qt = pp.tile([P, NQ], bf16, tag="qt")

            # ---- prologue: project Q^T, K^T, V from streamed x^T ----------
            for ch in range(NCH):
                xt_t = [sp.tile([P, CH], bf16, tag=f"xt{k}", name=f"xt{k}") for k in range(FK)]
                for k in range(FK):
                    nc.gpsimd.dma_start(
                        out=xt_t[k][:],
                        in_=xt[k * P:(k + 1) * P, ch * CH:(ch + 1) * CH])
                g, off = ch // 4, (ch % 4) * CH
                # K^T chunk [md, CH]
                pk = ps_sc.tile([P, CH], f32, tag="sc", name="pk")
                for k in range(FK):
                    nc.tensor.matmul(pk[:], wk_t[k][:], xt_t[k][:],
                                     start=(k == 0), stop=(k == FK - 1))
                nc.scalar.activation(ktg[g][:, off:off + CH], pk[:],
                                     AF.Identity, bias=bk_t[:], scale=1.0)
                # Q^T chunk (the first NQ columns are this core's queries)
                if ch < NQ // CH:
                    pq = ps_sc.tile([P, CH], f32, tag="sc", name="pq")
                    for k in range(FK):
                        nc.tensor.matmul(pq[:], wq_t[k][:], xt_t[k][:],
                                         start=(k == 0), stop=(k == FK - 1))
                    nc.scalar.activation(qt[:, ch * CH:(ch + 1) * CH], pq[:],
                                         AF.Identity, bias=bq_t[:], scale=1.0)
                # V tiles [j, f]
                for js in range(CH // JT):
                    jt_g = ch * (CH // JT) + js
                    voff = (jt_g % GK) * F
                    pv = ps_sc.tile([P, F], f32, tag="sc", name="pv")
                    for k in range(FK):
                        nc.tensor.matmul(
                            pv[:], xt_t[k][:, js * JT:(js + 1) * JT], wv_t[k][:],
                            start=(k == 0), stop=(k == FK - 1))
                    nc.vector.tensor_tensor(
                        vg[jt_g // GK][:, voff:voff + F], pv[:], bv_r[:], ALU.add)

            # ---- attention: per q-block sweep over all keys ---------------
            # j-tiles processed in PAIRS: exp -> fp8, V-matmuls in DoubleRow
            # (contraction 256 per MM at 0.5 cyc/row).
            for qb in range(NQB):
                q_sl = qt[:, qb * QB:(qb + 1) * QB]
                po = [ps_o.tile([P, QB], f32, tag="oacc", name="oacc") for _ in range(FK)]
                acc = [wkp.tile([P, QB], f32, tag="acc", bufs=4, name=f"acc{a}")
                       for a in range(2)]

                def scores(jt_i):
                    g, r = jt_i // GK, jt_i % GK
                    psc = ps_sc.tile([P, QB], f32, tag="sc", name="psc")
                    nc.tensor.matmul(psc[:], ktg[g][:, r * JT:(r + 1) * JT],
                                     q_sl, start=True, stop=True)
                    return psc

                pending = {j: scores(j) for j in range(3)}
                NP2 = NJT // 2
                for p_i in range(NP2):
                    jt0 = 2 * p_i
                    g, r0 = jt0 // GK, jt0 % GK
                    etp = wkp.tile([P, 2 * QB], fp8, tag="et", bufs=4)
                    for h in range(2):
                        psc = pending.pop(jt0 + h)
                        nc.scalar.activation(etp[:, h * QB:(h + 1) * QB], psc[:],
                                             AF.Exp, scale=SCALE)
                        nxt = jt0 + h + 3
                        if nxt < NJT:
                            pending[nxt] = scores(nxt)
                        # row-sum accumulation on DVE (two chains)
                        if p_i == 0:
                            nc.vector.tensor_copy(acc[h][:],
                                                  etp[:, h * QB:(h + 1) * QB])
                        else:
                            nc.vector.tensor_tensor(acc[h][:], acc[h][:],
                                                    etp[:, h * QB:(h + 1) * QB],
                                                    ALU.add)
                    et3 = etp.rearrange("p (h q) -> p h q", h=2)
                    vg4 = vg[g].rearrange("p (t h f) -> p t h f", h=2, f=F)
                    for ft in range(FK):
                        nc.tensor.matmul(
                            po[ft][:],
                            vg4[:, r0 // 2, :, ft * P:(ft + 1) * P],
                            et3, start=(p_i == 0), stop=(p_i == NP2 - 1),
                            perf_mode=DR)
                # epilogue: per-partition 1/sums, project, scale+bias, store
                ot = wkp.tile([P, FK * QB], bf16, tag="ot", bufs=1)
                for ft in range(FK):
                    nc.vector.tensor_copy(ot[:, ft * QB:(ft + 1) * QB], po[ft][:])
                recip_p = wkp.tile([P, QB // P], f32, tag="recipp", bufs=2)
                for qs in range(QB // P):
                    pt = ps_sf.tile([P, 1], f32, tag="sf", name="pt")
                    nc.tensor.matmul(pt[:], acc[0][:, qs * P:(qs + 1) * P],
                                     ones_f[:], start=True, stop=False)
                    nc.tensor.matmul(pt[:], acc[1][:, qs * P:(qs + 1) * P],
                                     ones_f[:], start=False, stop=True)
                    nc.vector.reciprocal(recip_p[:, qs:qs + 1], pt[:])
                    pf = ps_sf.tile([P, F], f32, tag="sf", name="pf")
                    for ft in range(FK):
                        nc.tensor.matmul(
                            pf[:], ot[:, ft * QB + qs * P:ft * QB + (qs + 1) * P],
                            wo_t[ft][:], start=(ft == 0), stop=(ft == FK - 1))
                    out_t = wkp.tile([P, F], f32, tag="outt", bufs=2)
                    nc.vector.scalar_tensor_tensor(
                        out_t[:], pf[:], recip_p[:, qs:qs + 1], bo_r[:],
                        ALU.mult, ALU.add)
                    row0 = qb * QB + qs * P
                    nc.sync.dma_start(out=out[row0:row0 + P, :], in_=out_t[:])

    nc.compile()
    return nc


_CACHED = {}


def _get_nc():
    if "nc" not in _CACHED:
        _CACHED["nc"] = _build()
    return _CACHED["nc"]


def kernel(x, Wq, bq, Wk, bk, Wv, bv, Wo, bo):
    from concourse.bass_utils import run_bass_kernel_spmd

    x = np.asarray(x, dtype=np.float32)
    xt_full = np.ascontiguousarray(x.T)                      # [F, N] f32
    wq_b = np.asarray(Wq, dtype=np.float32).astype(_BF16)
    wk_b = np.asarray(Wk, dtype=np.float32).astype(_BF16)
    wv_b = np.asarray(Wv, dtype=np.float32).astype(_BF16)
    wo_b = np.asarray(Wo, dtype=np.float32).astype(_BF16)
    bq_h = np.asarray(bq, dtype=np.float32).reshape(MD, 1)
    bk_h = np.asarray(bk, dtype=np.float32).reshape(MD, 1)
    bv_h = np.asarray(bv, dtype=np.float32).reshape(1, F)
    bo_h = np.asarray(bo, dtype=np.float32).reshape(1, F)

    in_maps = []
    for c in range(NCORES):
        s = c * NQ
        xt_rot = np.concatenate([xt_full[:, s:], xt_full[:, :s]], axis=1)
        in_maps.append({
            "xt": np.ascontiguousarray(xt_rot).astype(_BF16),
            "wq": wq_b, "wk": wk_b, "wv": wv_b, "wo": wo_b,
            "bq": bq_h, "bk": bk_h, "bv": bv_h, "bo": bo_h,
        })

    nc = _get_nc()
    res = run_bass_kernel_spmd(nc, in_maps, core_ids=list(range(NCORES)))
    return np.concatenate(
        [res.results[c]["out"] for c in range(NCORES)], axis=0)


def run_traced(x, Wq, bq, Wk, bk, Wv, bv, Wo, bo):
    """Like kernel() but with NTFF tracing; returns (output, exec_time_ns)."""
    from concourse.bass_utils import run_bass_kernel_spmd

    try:
        import ntff_shim
        ntff_shim.install()
    except ImportError:
        pass
    x = np.asarray(x, dtype=np.float32)
    xt_full = np.ascontiguousarray(x.T)
    in_maps = []
    for c in range(NCORES):
        s = c * NQ
        xt_rot = np.concatenate([xt_full[:, s:], xt_full[:, :s]], axis=1)
        in_maps.append({
            "xt": np.ascontiguousarray(xt_rot).astype(_BF16),
            "wq": np.asarray(Wq, np.float32).astype(_BF16),
            "wk": np.asarray(Wk, np.float32).astype(_BF16),
            "wv": np.asarray(Wv, np.float32).astype(_BF16),
            "wo": np.asarray(Wo, np.float32).astype(_BF16),
            "bq": np.asarray(bq, np.float32).reshape(MD, 1),
            "bk": np.asarray(bk, np.float32).reshape(MD, 1),
            "bv": np.asarray(bv, np.float32).reshape(1, F),
            "bo": np.asarray(bo, np.float32).reshape(1, F),
        })
    nc = _get_nc()
    res = run_bass_kernel_spmd(nc, in_maps, core_ids=list(range(NCORES)),
                               trace=True)
    out = np.concatenate([res.results[c]["out"] for c in range(NCORES)], axis=0)
    return out, res.exec_time_ns


# revision 8
# speedup vs baseline: 2.1557x; 1.0321x over previous
"""Trainium2 Bass kernel for single-head attention (N=16384, F=512, M=128),
sequence-parallel over 8 NeuronCores.

Strategy (hardcoded, self-contained):
- Each core owns 2048 query rows. K/V projections are computed redundantly on
  every core (fp8 DoubleRow makes them cheap) -> no collectives.
- Host passes x^T in fp8 per core, rotated so the core's own query columns are
  always columns 0:2048 -> identical SPMD graph on all cores. Softmax sums are
  permutation-invariant over keys, so rotated K/V order is harmless.
- Projection weights are pre-scaled by 16 on the host so fp8e4m3 stays in its
  normal range; the 1/256 compensation folds into the exp() scale and Wo/16.
- bk drops out of softmax exactly; bv passes through the attention average
  unchanged, so the host folds it into bo' = bv @ Wo + bo.
- Scores are computed transposed (S^T = K @ Q^T, layout [j, q]) so the exp
  output E^T feeds V^T E directly with no transposes. E and V are fp8; the
  O-accumulation matmuls use DoubleRow (two key-tiles contracted per matmul).
- Softmax denominators: E tiles are accumulated elementwise on the Vector and
  GpSimd engines (split to keep both under the PE's pace), then reduced
  across partitions by tiny fp32 matmuls directly into per-partition [q,1]
  layout; 1/sum is applied after the (linear) output projection.
"""

import math
import sys

import numpy as np

for _p in ("/opt/trn_rl_repo", "/opt/pypackages"):
    if _p not in sys.path:
        sys.path.append(_p)

import ml_dtypes

N = 16384
F = 512
MD = 128
P = 128
NCORES = 8
NQ = N // NCORES      # 2048 query rows per core
QB = 512              # q-block (one PSUM bank of fp32)
NQB = NQ // QB        # 4
JT = 128              # j (key) tile
NJT = N // JT         # 128
FK = F // P           # 4 contraction tiles over features
CH = 512              # xt streaming chunk (j columns)
NCH = N // CH         # 32
GK = 16               # j-tiles per SBUF super-group
NG = NJT // GK        # 8
WS = 16.0             # host-side fp8 weight pre-scale
SCALE = 1.0 / math.sqrt(MD) / (WS * WS)

_BF16 = ml_dtypes.bfloat16
_FP8 = ml_dtypes.float8_e4m3fn


def _build():
    import concourse.bass as bass  # noqa: F401
    import concourse.tile as tile
    from concourse import bacc, mybir

    f32 = mybir.dt.float32
    bf16 = mybir.dt.bfloat16
    fp8 = mybir.dt.float8e4
    DR = mybir.MatmulPerfMode.DoubleRow
    AF = mybir.ActivationFunctionType
    ALU = mybir.AluOpType

    nc = bacc.Bacc("TRN2", target_bir_lowering=False, debug=False,
                   num_devices=NCORES)

    xt = nc.declare_dram_parameter("xt", [F, N], fp8, isOutput=False)
    wq = nc.declare_dram_parameter("wq", [F, MD], fp8, isOutput=False)
    wk = nc.declare_dram_parameter("wk", [F, MD], fp8, isOutput=False)
    wv = nc.declare_dram_parameter("wv", [F, F], fp8, isOutput=False)
    wo = nc.declare_dram_parameter("wo", [F, F], bf16, isOutput=False)
    bq = nc.declare_dram_parameter("bq", [MD, 1], f32, isOutput=False)
    bo = nc.declare_dram_parameter("bo", [1, F], f32, isOutput=False)
    out = nc.declare_dram_parameter("out", [NQ, F], f32, isOutput=True)

    with tile.TileContext(nc) as tc:
        with (
            tc.tile_pool(name="persist", bufs=1) as pp,
            tc.tile_pool(name="stream", bufs=3) as sp,
            tc.tile_pool(name="work", bufs=3) as wkp,
            tc.tile_pool(name="pssc", bufs=3, space="PSUM") as ps_sc,
            tc.tile_pool(name="pso", bufs=4, space="PSUM") as ps_o,
            tc.tile_pool(name="pssf", bufs=1, space="PSUM") as ps_sf,
        ):
            # ---- persistent constants (vector/scalar DMA queues so the
            # gpsimd xt stream is not serialized behind them) --------------
            wq_a = pp.tile([P, FK, MD], fp8, tag="wqa")
            wk_a = pp.tile([P, FK, MD], fp8, tag="wka")
            wv_a = pp.tile([P, FK, F], fp8, tag="wva")
            wo_t = [pp.tile([P, F], bf16, tag=f"wo{k}", name=f"wo{k}")
                    for k in range(FK)]
            for k in range(FK):
                nc.scalar.dma_start(out=wk_a[:, k, :], in_=wk[k * P:(k + 1) * P, :])
                nc.scalar.dma_start(out=wv_a[:, k, :], in_=wv[k * P:(k + 1) * P, :])
                nc.scalar.dma_start(out=wq_a[:, k, :], in_=wq[k * P:(k + 1) * P, :])
                nc.scalar.dma_start(out=wo_t[k][:], in_=wo[k * P:(k + 1) * P, :])
            bq_t = pp.tile([MD, 1], f32, tag="bq")
            nc.scalar.dma_start(out=bq_t[:], in_=bq[:])
            bo_r = pp.tile([P, F], f32, tag="bor")
            nc.scalar.dma_start(out=bo_r[:], in_=bo[:].to_broadcast((P, F)))
            ones_f = pp.tile([P, 1], f32, tag="ones")
            nc.vector.memset(ones_f[:], 1.0)

            # ---- persistent activations -----------------------------------
            ktg = [pp.tile([P, GK * JT], bf16, tag=f"ktg{g}", name=f"ktg{g}")
                   for g in range(NG)]
            vg = [pp.tile([P, GK * F], fp8, tag=f"vg{g}", name=f"vg{g}")
                  for g in range(NG)]
            qt = pp.tile([P, NQ], bf16, tag="qt")

            # ---- prologue: project Q^T, K^T, V (fp8 DoubleRow) ------------
            for ch in range(NCH):
                xtc = sp.tile([P, FK, CH], fp8, tag="xtc")
                for k in range(FK):
                    nc.gpsimd.dma_start(
                        out=xtc[:, k, :],
                        in_=xt[k * P:(k + 1) * P, ch * CH:(ch + 1) * CH])
                g, off = ch // 4, (ch % 4) * CH
                pk = ps_sc.tile([P, CH], f32, tag="sc", name="pk")
                for h in range(2):
                    nc.tensor.matmul(pk[:], wk_a[:, 2 * h:2 * h + 2, :],
                                     xtc[:, 2 * h:2 * h + 2, :],
                                     start=(h == 0), stop=(h == 1), perf_mode=DR)
                nc.scalar.copy(ktg[g][:, off:off + CH], pk[:])
                if ch < NQ // CH:
                    pq = ps_sc.tile([P, CH], f32, tag="sc", name="pq")
                    for h in range(2):
                        nc.tensor.matmul(pq[:], wq_a[:, 2 * h:2 * h + 2, :],
                                         xtc[:, 2 * h:2 * h + 2, :],
                                         start=(h == 0), stop=(h == 1),
                                         perf_mode=DR)
                    nc.scalar.activation(qt[:, ch * CH:(ch + 1) * CH], pq[:],
                                         AF.Identity, bias=bq_t[:], scale=1.0)
                for js in range(CH // JT):
                    jt_g = ch * (CH // JT) + js
                    voff = (jt_g % GK) * F
                    pv = ps_sc.tile([P, F], f32, tag="sc", name="pv")
                    for h in range(2):
                        nc.tensor.matmul(
                            pv[:], xtc[:, 2 * h:2 * h + 2, js * JT:(js + 1) * JT],
                            wv_a[:, 2 * h:2 * h + 2, :],
                            start=(h == 0), stop=(h == 1), perf_mode=DR)
                    nc.vector.tensor_copy(vg[jt_g // GK][:, voff:voff + F], pv[:])

            # ---- attention: per q-block sweep over all keys ---------------
            NP2 = NJT // 2
            for qb in range(NQB):
                q_sl = qt[:, qb * QB:(qb + 1) * QB]
                po = [ps_o.tile([P, QB], f32, tag="oacc", name="oacc")
                      for _ in range(FK)]
                acc_d = wkp.tile([P, 2 * QB], f32, tag="accd", bufs=2)
                acc_g = wkp.tile([P, 2 * QB], f32, tag="accg", bufs=2)

                def scores(jt_i):
                    g, r = jt_i // GK, jt_i % GK
                    psc = ps_sc.tile([P, QB], f32, tag="sc", name="psc")
                    nc.tensor.matmul(psc[:], ktg[g][:, r * JT:(r + 1) * JT],
                                     q_sl, start=True, stop=True)
                    return psc

                pending = {j: scores(j) for j in range(3)}
                seen = {"d": False, "g": False}
                for p_i in range(NP2):
                    jt0 = 2 * p_i
                    g, r0 = jt0 // GK, jt0 % GK
                    etp = wkp.tile([P, 2 * QB], fp8, tag="et", bufs=6)
                    for h in range(2):
                        psc = pending.pop(jt0 + h)
                        nc.scalar.activation(etp[:, h * QB:(h + 1) * QB], psc[:],
                                             AF.Exp, scale=SCALE)
                        nxt = jt0 + h + 3
                        if nxt < NJT:
                            pending[nxt] = scores(nxt)
                    # row-sum accumulation, split across DVE and GpSimd
                    if p_i % 8 < 5:
                        eng, acc, key = nc.vector, acc_d, "d"
                    else:
                        eng, acc, key = nc.gpsimd, acc_g, "g"
                    if not seen[key]:
                        eng.tensor_copy(acc[:], etp[:])
                        seen[key] = True
                    else:
                        eng.tensor_tensor(acc[:], acc[:], etp[:], ALU.add)
                    et3 = etp.rearrange("p (h q) -> p h q", h=2)
                    vg4 = vg[g].rearrange("p (t h f) -> p t h f", h=2, f=F)
                    for ft in range(FK):
                        nc.tensor.matmul(
                            po[ft][:],
                            vg4[:, r0 // 2, :, ft * P:(ft + 1) * P],
                            et3, start=(p_i == 0), stop=(p_i == NP2 - 1),
                            perf_mode=DR)
                # epilogue: 1/sums (per-partition), project, scale+bias, store
                ot = wkp.tile([P, FK * QB], bf16, tag="ot", bufs=1)
                for ft in range(FK):
                    nc.vector.tensor_copy(ot[:, ft * QB:(ft + 1) * QB], po[ft][:])
                recip_p = wkp.tile([P, QB // P], f32, tag="recipp", bufs=2)
                for qs in range(QB // P):
                    pt = ps_sf.tile([P, 1], f32, tag="sf", name="pt")
                    srcs = [acc_d[:, qs * P:(qs + 1) * P],
                            acc_d[:, QB + qs * P:QB + (qs + 1) * P],
                            acc_g[:, qs * P:(qs + 1) * P],
                            acc_g[:, QB + qs * P:QB + (qs + 1) * P]]
                    for si, s in enumerate(srcs):
                        nc.tensor.matmul(pt[:], s, ones_f[:],
                                         start=(si == 0), stop=(si == 3))
                    nc.vector.reciprocal(recip_p[:, qs:qs + 1], pt[:])
                    pf = ps_sf.tile([P, F], f32, tag="sf", name="pf")
                    for ft in range(FK):
                        nc.tensor.matmul(
                            pf[:], ot[:, ft * QB + qs * P:ft * QB + (qs + 1) * P],
                            wo_t[ft][:], start=(ft == 0), stop=(ft == FK - 1))
                    out_t = wkp.tile([P, F], f32, tag="outt", bufs=2)
                    nc.vector.scalar_tensor_tensor(
                        out_t[:], pf[:], recip_p[:, qs:qs + 1], bo_r[:],
                        ALU.mult, ALU.add)
                    row0 = qb * QB + qs * P
                    nc.sync.dma_start(out=out[row0:row0 + P, :], in_=out_t[:])

    nc.compile()
    return nc


_CACHED = {}


def _get_nc():
    if "nc" not in _CACHED:
        _CACHED["nc"] = _build()
    return _CACHED["nc"]


def _make_in_maps(x, Wq, bq, Wk, bk, Wv, bv, Wo, bo):
    x = np.asarray(x, dtype=np.float32)
    xt_full = np.ascontiguousarray(x.T)                     # [F, N] f32
    wq_8 = (WS * np.asarray(Wq, np.float32)).astype(_FP8)
    wk_8 = (WS * np.asarray(Wk, np.float32)).astype(_FP8)
    wv_8 = (WS * np.asarray(Wv, np.float32)).astype(_FP8)
    wo_b = (np.asarray(Wo, np.float32) / WS).astype(_BF16)
    bq_h = (WS * np.asarray(bq, np.float32)).reshape(MD, 1).astype(np.float32)
    bo_p = (np.asarray(bv, np.float64) @ np.asarray(Wo, np.float64)
            + np.asarray(bo, np.float64)).astype(np.float32).reshape(1, F)

    in_maps = []
    for c in range(NCORES):
        s = c * NQ
        xt_rot = np.concatenate([xt_full[:, s:], xt_full[:, :s]], axis=1)
        in_maps.append({
            "xt": np.ascontiguousarray(xt_rot).astype(_FP8),
            "wq": wq_8, "wk": wk_8, "wv": wv_8, "wo": wo_b,
            "bq": bq_h, "bo": bo_p,
        })
    return in_maps


def kernel(x, Wq, bq, Wk, bk, Wv, bv, Wo, bo):
    from concourse.bass_utils import run_bass_kernel_spmd

    in_maps = _make_in_maps(x, Wq, bq, Wk, bk, Wv, bv, Wo, bo)
    nc = _get_nc()
    res = run_bass_kernel_spmd(nc, in_maps, core_ids=list(range(NCORES)))
    return np.concatenate(
        [res.results[c]["out"] for c in range(NCORES)], axis=0)


def run_traced(x, Wq, bq, Wk, bk, Wv, bv, Wo, bo):
    """Like kernel() but with NTFF tracing; returns (output, exec_time_ns)."""
    from concourse.bass_utils import run_bass_kernel_spmd

    try:
        import ntff_shim
        ntff_shim.install()
    except ImportError:
        pass
    in_maps = _make_in_maps(x, Wq, bq, Wk, bk, Wv, bv, Wo, bo)
    nc = _get_nc()
    res = run_bass_kernel_spmd(nc, in_maps, core_ids=list(range(NCORES)),
                               trace=True)
    out = np.concatenate([res.results[c]["out"] for c in range(NCORES)], axis=0)
    return out, res.exec_time_ns
